# revision 28
# baseline (speedup 1.0000x reference)
"""Trainium2 Bass kernel for a gated LoRA adapter layer (MoE-style routing).

Computes, for x:(8,2048,4096) f32, type_weight:(8,2048) f32,
lora_A:(4096,64) f32, lora_B:(64,4096) f32:

    out = type_weight[..., None] * ((x @ lora_A) @ lora_B) * 2.0

Routing insight: ~50% of tokens have type_weight == 0 and contribute an
exactly-zero output row.  The host compacts the nonzero tokens (the
"router"), folds the gate into x (x_row * 2*tw), pre-transposes so the
contraction dim lands on partitions, and casts everything to bf16.  The
8 cores each run a dense (x.T-major) LoRA on exactly 1024 tokens (two
512-token stages); the device capacity is 8*1024 = 8192 global tokens —
right at the mean nonzero count — and any overflow tokens (mean ~25,
std ~64 for Bernoulli(0.5) gates) are computed exactly on the host in
f32 numpy.  Outputs are stored bf16 and scattered into the
zero-initialized full f32 result.

Device pipeline per core:
  - mm1: t.T = sum_dt A[dt].T @ xT[dt], with A's columns duplicated so the
    [128, 512] PSUM result holds t.T on partitions 0-63 AND 64-127.
  - mm2: out row-blocks via PAIRED matmuls in disjoint PE row groups
    (rows 0-63 / 64-127, K=64 each) -> 2 concurrent MMs per issue, into
    one [128, 1024] two-bank PSUM tile drained by a single copy.
  - mm1 of stage 1 is interleaved between mm2 slots of stage 0 so orows
    start flowing early and the PE fills the copy-drain time.
  - B is duplicated to partitions 64-127 on-device (SBUF->SBUF DMA).
"""

import numpy as np
import ml_dtypes

BF16 = ml_dtypes.bfloat16

B_CORES = 8
S = 2048
D = 4096
R = 64
LORA_SCALING = 128.0 / 64.0

T_STAGE = 512
N_STAGES = 2
S_PAD = T_STAGE * N_STAGES  # 1024 per-core device capacity
N_DT = D // 128             # 32 d-tiles
N_DC = D // 512             # 8 output column chunks
N_ST = T_STAGE // 128       # 4 output row blocks per stage (2 pairs)
HOST_OVERFLOW_MAX = 2048    # beyond this, loop more device runs

_CACHE = {}

# osb tiles are [128, 2, D] pairs (16KB/partition); ps_o tiles are single
# PSUM banks ([128, 512] f32) - 6 bufs = 3 pair-slots of pipeline depth.
OPTS = {
    "x_bufs": 2,
    "osb_bufs": 5,
    "ps_t_bufs": 2,
    "ps_o_bufs": 6,
}


def _build_bass():
    import concourse.tile as tile
    from concourse import bacc, mybir

    nc = bacc.Bacc(
        "TRN2",
        debug=False,
        enable_asserts=False,
        target_bir_lowering=False,
        num_devices=B_CORES,
    )

    f32 = mybir.dt.float32
    bf16 = mybir.dt.bfloat16

    # Host-prepped layouts (see _prep_core / _prep_weights):
    #   x:  [128, 32*S_PAD]  stage-major [p][j][dt][s], d = dt*128 + p
    #   a:  [128, N_DT * R]  = [p][dt][r]
    #   b:  [R, D]
    x_d = nc.dram_tensor("x", [128, N_DT * S_PAD], bf16, kind="ExternalInput").ap()
    a_d = nc.dram_tensor("lora_a", [128, N_DT * R], bf16, kind="ExternalInput").ap()
    b_d = nc.dram_tensor("lora_b", [2 * R, D], bf16, kind="ExternalInput").ap()
    out_d = nc.dram_tensor("out", [S_PAD, D], bf16, kind="ExternalOutput").ap()

    with tile.TileContext(nc) as tc:
        with (
            tc.tile_pool(name="consts", bufs=1) as consts,
            tc.tile_pool(name="xsb", bufs=OPTS["x_bufs"]) as xsb,
            tc.tile_pool(name="ttp", bufs=2) as ttp,
            tc.tile_pool(name="osb", bufs=OPTS["osb_bufs"]) as osb,
            tc.tile_pool(name="ps_t", bufs=OPTS["ps_t_bufs"], space="PSUM") as ps_t,
            tc.tile_pool(name="ps_o", bufs=OPTS["ps_o_bufs"], space="PSUM") as ps_o,
        ):
            # A with duplicated columns: a_sb[p, dt, 0:64] == a_sb[p, dt, 64:128]
            # == A[dt*128+p, :].  Load once, duplicate with DVE copies.
            a_tmp = consts.tile([128, N_DT, R], bf16)
            nc.sync.dma_start(a_tmp[:], a_d.rearrange("p (dt r) -> p dt r", r=R))
            a_sb = consts.tile([128, N_DT, 2 * R], bf16)
            nc.vector.tensor_copy(a_sb[:, :, 0:R], a_tmp[:])
            nc.vector.tensor_copy(a_sb[:, :, R : 2 * R], a_tmp[:])

            # B arrives host-duplicated to 128 partitions (an on-device
            # SBUF->SBUF SWDGE duplicate measured ~10us late on the critical
            # path — every B-row-group matmul gated on it).
            b_sb = consts.tile([128, D], bf16)

            # x stage loads in quarters so mm1 chases the FIFO-ordered loads;
            # B slots in after stage 0 so mm2(0) can start.
            xts = []
            for j in range(N_STAGES):
                xt = xsb.tile([128, N_DT, T_STAGE], bf16, name=f"xt{j}", tag="xt")
                for h in range(4):
                    off = (j * N_DT + h * 8) * T_STAGE
                    src = x_d[:, off : off + 8 * T_STAGE].rearrange(
                        "p (dt s) -> p dt s", s=T_STAGE
                    )
                    nc.sync.dma_start(xt[:, h * 8 : (h + 1) * 8, :], src)
                if j == 0:
                    nc.sync.dma_start(b_sb[:], b_d)
                xts.append(xt)

            def emit_mm1(j, ps, dt):
                # t.T (duplicated over both partition halves) accumulated f32.
                nc.tensor.matmul(
                    ps[:],
                    lhsT=a_sb[:, dt, :],
                    rhs=xts[j][:, dt, :],
                    start=(dt == 0),
                    stop=(dt == N_DT - 1),
                )

            def emit_mm2_stage(j, ttj, interleave):
                """mm2 slots for stage j; optionally interleave (fn per slot)."""
                for q in range(N_ST // 2):
                    stA, stB = 2 * q, 2 * q + 1
                    orow2 = osb.tile(
                        [128, 2, D], bf16, name=f"or2_{j}_{q}", tag="orow"
                    )
                    for dc in range(N_DC):
                        cs = slice(dc * 512, (dc + 1) * 512)
                        psoA = ps_o.tile([128, 512], f32, name="psoA", tag="pso")
                        nc.tensor.matmul(
                            psoA[:],
                            lhsT=ttj[0:R, stA * 128 : (stA + 1) * 128],
                            rhs=b_sb[0:R, cs],
                            start=True,
                            stop=True,
                        )
                        psoB = ps_o.tile([128, 512], f32, name="psoB", tag="pso")
                        nc.tensor.matmul(
                            psoB[:],
                            lhsT=ttj[R : 2 * R, stB * 128 : (stB + 1) * 128],
                            rhs=b_sb[R : 2 * R, cs],
                            start=True,
                            stop=True,
                        )
                        if interleave is not None:
                            interleave(q * N_DC + dc)
                        nc.vector.tensor_copy(orow2[:, 0, cs], psoA[:])
                        nc.scalar.copy(orow2[:, 1, cs], psoB[:])
                    r0 = (j * N_ST + stA) * 128
                    nc.scalar.dma_start(out_d[r0 : r0 + 128, :], orow2[:, 0, :])
                    nc.scalar.dma_start(out_d[r0 + 128 : r0 + 256, :], orow2[:, 1, :])

            # mm1(0) dense, then mm2(0) with mm1(1) interleaved 2-per-slot
            # (the scheduler fills copy-drain bubbles with mm1 work), then
            # mm2(1) plain (copy-paced; PE throttle is irrelevant there).
            ps0 = ps_t.tile([128, T_STAGE], f32, name="psmm1_0", tag="mm1ps")
            for dt in range(N_DT):
                emit_mm1(0, ps0, dt)
            tt0 = ttp.tile([128, T_STAGE], bf16, name="tt0", tag="tt")
            nc.vector.tensor_copy(tt0[:], ps0[:])

            ps1 = ps_t.tile([128, T_STAGE], f32, name="psmm1_1", tag="mm1ps")

            def ilv(slot):
                for dt in (2 * slot, 2 * slot + 1):
                    if dt < N_DT:
                        emit_mm1(1, ps1, dt)

            emit_mm2_stage(0, tt0, ilv)

            tt1 = ttp.tile([128, T_STAGE], bf16, name="tt1", tag="tt")
            nc.vector.tensor_copy(tt1[:], ps1[:])
            emit_mm2_stage(1, tt1, None)

    nc.compile()
    return nc


def get_bass():
    if "nc" not in _CACHE:
        _CACHE["nc"] = _build_bass()
    return _CACHE["nc"]


def _prep_weights(lora_A, lora_B):
    a = np.asarray(lora_A, dtype=np.float32).astype(BF16)
    # [D, R] -> [p][dt][r] with d = dt*128 + p
    a_p = np.ascontiguousarray(a.reshape(N_DT, 128, R).transpose(1, 0, 2)).reshape(
        128, N_DT * R
    )
    b = np.asarray(lora_B, dtype=np.float32).astype(BF16)
    b_p = np.ascontiguousarray(np.concatenate([b, b], axis=0))  # [2R, D]
    return a_p, b_p


def _prep_core(x2, scale, ids):
    """Gather + gate-fold + pad + transpose one core's tokens.

    Returns [128, N_DT*S_PAD] bf16, stage-major [p][j][dt][s]."""
    n = len(ids)
    xsb = np.zeros((S_PAD, D), dtype=BF16)
    if n:
        xsb[:n] = (x2[ids] * scale[:, None]).astype(BF16)
    blk = xsb.reshape(N_STAGES, T_STAGE, N_DT, 128).transpose(3, 0, 2, 1)
    return np.ascontiguousarray(blk).reshape(128, N_DT * S_PAD)


def _make_chunk_in_maps(x2, twf, idx_chunk, a_p, b_p):
    splits = np.array_split(idx_chunk, B_CORES)
    in_maps = []
    for ids in splits:
        scale = LORA_SCALING * twf[ids]
        in_maps.append(
            {
                "x": _prep_core(x2, scale, ids),
                "lora_a": a_p,
                "lora_b": b_p,
            }
        )
    return in_maps, splits


def make_in_maps(x, type_weight, lora_A, lora_B):
    """First-chunk in_maps (what kernel() runs on the device)."""
    x2 = np.asarray(x, dtype=np.float32).reshape(B_CORES * S, D)
    twf = np.asarray(type_weight, dtype=np.float32).reshape(B_CORES * S)
    idx = np.flatnonzero(twf)[: B_CORES * S_PAD]
    a_p, b_p = _prep_weights(lora_A, lora_B)
    in_maps, _ = _make_chunk_in_maps(x2, twf, idx, a_p, b_p)
    return in_maps


def kernel(x, type_weight, lora_A, lora_B):
    from concourse.bass_utils import run_bass_kernel_spmd

    x2 = np.asarray(x, dtype=np.float32).reshape(B_CORES * S, D)
    twf = np.asarray(type_weight, dtype=np.float32).reshape(B_CORES * S)
    out = np.zeros((B_CORES * S, D), dtype=np.float32)

    idx = np.flatnonzero(twf)
    cap = B_CORES * S_PAD
    pos = 0
    if len(idx):
        # Device runs on chunks of `cap` tokens while the remainder is large;
        # the final small overflow (mean ~25 tokens for 50%-sparse gates) is
        # computed exactly on the host instead of paying another device run.
        a_p = b_p = None
        while len(idx) - pos > HOST_OVERFLOW_MAX or (pos == 0 and len(idx) - pos > 0):
            chunk = idx[pos : pos + cap]
            if a_p is None:
                nc = get_bass()
                a_p, b_p = _prep_weights(lora_A, lora_B)
            in_maps, splits = _make_chunk_in_maps(x2, twf, chunk, a_p, b_p)
            res = run_bass_kernel_spmd(nc, in_maps, list(range(B_CORES)))
            for i, ids in enumerate(splits):
                if len(ids):
                    out[ids] = res.results[i]["out"][: len(ids)].astype(np.float32)
            pos += len(chunk)

    if pos < len(idx):
        ids = idx[pos:]
        a32 = np.asarray(lora_A, dtype=np.float32)
        b32 = np.asarray(lora_B, dtype=np.float32)
        xs = x2[ids] * (LORA_SCALING * twf[ids])[:, None]
        out[ids] = (xs @ a32) @ b32

    return out.reshape(B_CORES, S, D)


if __name__ == "__main__":
    nc = get_bass()
    print("built + compiled ok")


# revision 29
# speedup vs baseline: 1.0837x; 1.0837x over previous
"""Trainium2 Bass kernel for a gated LoRA adapter layer (MoE-style routing).

Computes, for x:(8,2048,4096) f32, type_weight:(8,2048) f32,
lora_A:(4096,64) f32, lora_B:(64,4096) f32:

    out = type_weight[..., None] * ((x @ lora_A) @ lora_B) * 2.0

Routing insight: ~50% of tokens have type_weight == 0 and contribute an
exactly-zero output row.  The host compacts the nonzero tokens (the
"router"), folds the gate into x (x_row * 2*tw), pre-transposes so the
contraction dim lands on partitions, and casts everything to bf16.  The
8 cores each run a dense (x.T-major) LoRA on exactly 1024 tokens (two
512-token stages); the device capacity is 8*1024 = 8192 global tokens —
right at the mean nonzero count — and any overflow tokens (mean ~25,
std ~64 for Bernoulli(0.5) gates) are computed exactly on the host in
f32 numpy.  Outputs are stored bf16 and scattered into the
zero-initialized full f32 result.

Device pipeline per core:
  - mm1: t.T = sum_dt A[dt].T @ xT[dt], with A's columns duplicated so the
    [128, 512] PSUM result holds t.T on partitions 0-63 AND 64-127.
  - mm2: out row-blocks via PAIRED matmuls in disjoint PE row groups
    (rows 0-63 / 64-127, K=64 each) -> 2 concurrent MMs per issue, into
    one [128, 1024] two-bank PSUM tile drained by a single copy.
  - mm1 of stage 1 is interleaved between mm2 slots of stage 0 so orows
    start flowing early and the PE fills the copy-drain time.
  - B is duplicated to partitions 64-127 on-device (SBUF->SBUF DMA).
"""

import numpy as np
import ml_dtypes

BF16 = ml_dtypes.bfloat16

B_CORES = 8
S = 2048
D = 4096
R = 64
LORA_SCALING = 128.0 / 64.0

T_STAGE = 512
N_STAGES = 2
S_PAD = T_STAGE * N_STAGES  # 1024 per-core device capacity
N_DT = D // 128             # 32 d-tiles
N_DC = D // 512             # 8 output column chunks
N_ST = T_STAGE // 128       # 4 output row blocks per stage (2 pairs)
HOST_OVERFLOW_MAX = 2048    # beyond this, loop more device runs

_CACHE = {}

# osb tiles are [128, 2, D] pairs (16KB/partition); ps_o tiles are single
# PSUM banks ([128, 512] f32) - 6 bufs = 3 pair-slots of pipeline depth.
OPTS = {
    "x_bufs": 2,
    "osb_bufs": 5,
    "ps_t_bufs": 2,
    "ps_o_bufs": 6,
}


def _build_bass():
    import concourse.tile as tile
    from concourse import bacc, mybir

    nc = bacc.Bacc(
        "TRN2",
        debug=False,
        enable_asserts=False,
        target_bir_lowering=False,
        num_devices=B_CORES,
    )

    f32 = mybir.dt.float32
    bf16 = mybir.dt.bfloat16

    # Host-prepped layouts (see _prep_core / _prep_weights):
    #   x:  [128, 32*S_PAD]  stage-major [p][j][dt][s], d = dt*128 + p
    #   a:  [128, N_DT * R]  = [p][dt][r]
    #   b:  [R, D]
    x_d = nc.dram_tensor("x", [128, N_DT * S_PAD], bf16, kind="ExternalInput").ap()
    a_d = nc.dram_tensor("lora_a", [128, N_DT * R], bf16, kind="ExternalInput").ap()
    b_d = nc.dram_tensor("lora_b", [2 * R, D], bf16, kind="ExternalInput").ap()
    out_d = nc.dram_tensor("out", [S_PAD, D], bf16, kind="ExternalOutput").ap()

    with tile.TileContext(nc) as tc:
        with (
            tc.tile_pool(name="consts", bufs=1) as consts,
            tc.tile_pool(name="xsb", bufs=OPTS["x_bufs"]) as xsb,
            tc.tile_pool(name="ttp", bufs=2) as ttp,
            tc.tile_pool(name="osb", bufs=OPTS["osb_bufs"]) as osb,
            tc.tile_pool(name="ps_t", bufs=OPTS["ps_t_bufs"], space="PSUM") as ps_t,
            tc.tile_pool(name="ps_o", bufs=OPTS["ps_o_bufs"], space="PSUM") as ps_o,
        ):
            # A with duplicated columns: a_sb[p, dt, 0:64] == a_sb[p, dt, 64:128]
            # == A[dt*128+p, :].  Load once, duplicate with DVE copies.
            a_tmp = consts.tile([128, N_DT, R], bf16)
            nc.sync.dma_start(a_tmp[:], a_d.rearrange("p (dt r) -> p dt r", r=R))
            a_sb = consts.tile([128, N_DT, 2 * R], bf16)
            nc.vector.tensor_copy(a_sb[:, :, 0:R], a_tmp[:])
            nc.vector.tensor_copy(a_sb[:, :, R : 2 * R], a_tmp[:])

            # B arrives host-duplicated to 128 partitions (an on-device
            # SBUF->SBUF SWDGE duplicate measured ~10us late on the critical
            # path — every B-row-group matmul gated on it).
            b_sb = consts.tile([128, D], bf16)

            # x stage loads in quarters so mm1 chases the FIFO-ordered loads;
            # B slots in after stage 0 so mm2(0) can start.
            xts = []
            for j in range(N_STAGES):
                xt = xsb.tile([128, N_DT, T_STAGE], bf16, name=f"xt{j}", tag="xt")
                for h in range(4):
                    off = (j * N_DT + h * 8) * T_STAGE
                    src = x_d[:, off : off + 8 * T_STAGE].rearrange(
                        "p (dt s) -> p dt s", s=T_STAGE
                    )
                    nc.sync.dma_start(xt[:, h * 8 : (h + 1) * 8, :], src)
                if j == 0:
                    nc.sync.dma_start(b_sb[:], b_d)
                xts.append(xt)

            def emit_mm1(j, ps, dt):
                # t.T (duplicated over both partition halves) accumulated f32.
                nc.tensor.matmul(
                    ps[:],
                    lhsT=a_sb[:, dt, :],
                    rhs=xts[j][:, dt, :],
                    start=(dt == 0),
                    stop=(dt == N_DT - 1),
                )

            def emit_mm2_stage(j, ttj, interleave):
                """mm2 slots for stage j; optionally interleave (fn per slot)."""
                for q in range(N_ST // 2):
                    stA, stB = 2 * q, 2 * q + 1
                    orow2 = osb.tile(
                        [128, 2, D], bf16, name=f"or2_{j}_{q}", tag="orow"
                    )
                    r0 = (j * N_ST + stA) * 128
                    for dc in range(N_DC):
                        cs = slice(dc * 512, (dc + 1) * 512)
                        psoA = ps_o.tile([128, 512], f32, name="psoA", tag="pso")
                        nc.tensor.matmul(
                            psoA[:],
                            lhsT=ttj[0:R, stA * 128 : (stA + 1) * 128],
                            rhs=b_sb[0:R, cs],
                            start=True,
                            stop=True,
                        )
                        psoB = ps_o.tile([128, 512], f32, name="psoB", tag="pso")
                        nc.tensor.matmul(
                            psoB[:],
                            lhsT=ttj[R : 2 * R, stB * 128 : (stB + 1) * 128],
                            rhs=b_sb[R : 2 * R, cs],
                            start=True,
                            stop=True,
                        )
                        if interleave is not None:
                            interleave(q * N_DC + dc)
                        nc.vector.tensor_copy(orow2[:, 0, cs], psoA[:])
                        nc.scalar.copy(orow2[:, 1, cs], psoB[:])
                        # Store each 1024-col pair as soon as its drains land,
                        # split across both DMA rings (ACT + the SP ring that
                        # sits idle once loads finish) so the store stream never
                        # waits for a full row-block and the tail store shrinks.
                        if dc % 2 == 1:
                            cs2 = slice((dc - 1) * 512, (dc + 1) * 512)
                            nc.scalar.dma_start(
                                out_d[r0 : r0 + 128, cs2], orow2[:, 0, cs2]
                            )
                            nc.sync.dma_start(
                                out_d[r0 + 128 : r0 + 256, cs2], orow2[:, 1, cs2]
                            )

            # mm1(0) dense, then mm2(0) with mm1(1) interleaved 2-per-slot
            # (the scheduler fills copy-drain bubbles with mm1 work), then
            # mm2(1) plain (copy-paced; PE throttle is irrelevant there).
            ps0 = ps_t.tile([128, T_STAGE], f32, name="psmm1_0", tag="mm1ps")
            for dt in range(N_DT):
                emit_mm1(0, ps0, dt)
            tt0 = ttp.tile([128, T_STAGE], bf16, name="tt0", tag="tt")
            nc.vector.tensor_copy(tt0[:], ps0[:])

            ps1 = ps_t.tile([128, T_STAGE], f32, name="psmm1_1", tag="mm1ps")

            def ilv(slot):
                for dt in (2 * slot, 2 * slot + 1):
                    if dt < N_DT:
                        emit_mm1(1, ps1, dt)

            emit_mm2_stage(0, tt0, ilv)

            tt1 = ttp.tile([128, T_STAGE], bf16, name="tt1", tag="tt")
            nc.vector.tensor_copy(tt1[:], ps1[:])
            emit_mm2_stage(1, tt1, None)

    nc.compile()
    return nc


def get_bass():
    if "nc" not in _CACHE:
        _CACHE["nc"] = _build_bass()
    return _CACHE["nc"]


def _prep_weights(lora_A, lora_B):
    a = np.asarray(lora_A, dtype=np.float32).astype(BF16)
    # [D, R] -> [p][dt][r] with d = dt*128 + p
    a_p = np.ascontiguousarray(a.reshape(N_DT, 128, R).transpose(1, 0, 2)).reshape(
        128, N_DT * R
    )
    b = np.asarray(lora_B, dtype=np.float32).astype(BF16)
    b_p = np.ascontiguousarray(np.concatenate([b, b], axis=0))  # [2R, D]
    return a_p, b_p


def _prep_core(x2, scale, ids):
    """Gather + gate-fold + pad + transpose one core's tokens.

    Returns [128, N_DT*S_PAD] bf16, stage-major [p][j][dt][s]."""
    n = len(ids)
    xsb = np.zeros((S_PAD, D), dtype=BF16)
    if n:
        xsb[:n] = (x2[ids] * scale[:, None]).astype(BF16)
    blk = xsb.reshape(N_STAGES, T_STAGE, N_DT, 128).transpose(3, 0, 2, 1)
    return np.ascontiguousarray(blk).reshape(128, N_DT * S_PAD)


def _make_chunk_in_maps(x2, twf, idx_chunk, a_p, b_p):
    splits = np.array_split(idx_chunk, B_CORES)
    in_maps = []
    for ids in splits:
        scale = LORA_SCALING * twf[ids]
        in_maps.append(
            {
                "x": _prep_core(x2, scale, ids),
                "lora_a": a_p,
                "lora_b": b_p,
            }
        )
    return in_maps, splits


def make_in_maps(x, type_weight, lora_A, lora_B):
    """First-chunk in_maps (what kernel() runs on the device)."""
    x2 = np.asarray(x, dtype=np.float32).reshape(B_CORES * S, D)
    twf = np.asarray(type_weight, dtype=np.float32).reshape(B_CORES * S)
    idx = np.flatnonzero(twf)[: B_CORES * S_PAD]
    a_p, b_p = _prep_weights(lora_A, lora_B)
    in_maps, _ = _make_chunk_in_maps(x2, twf, idx, a_p, b_p)
    return in_maps


def kernel(x, type_weight, lora_A, lora_B):
    from concourse.bass_utils import run_bass_kernel_spmd

    x2 = np.asarray(x, dtype=np.float32).reshape(B_CORES * S, D)
    twf = np.asarray(type_weight, dtype=np.float32).reshape(B_CORES * S)
    out = np.zeros((B_CORES * S, D), dtype=np.float32)

    idx = np.flatnonzero(twf)
    cap = B_CORES * S_PAD
    pos = 0
    if len(idx):
        # Device runs on chunks of `cap` tokens while the remainder is large;
        # the final small overflow (mean ~25 tokens for 50%-sparse gates) is
        # computed exactly on the host instead of paying another device run.
        a_p = b_p = None
        while len(idx) - pos > HOST_OVERFLOW_MAX or (pos == 0 and len(idx) - pos > 0):
            chunk = idx[pos : pos + cap]
            if a_p is None:
                nc = get_bass()
                a_p, b_p = _prep_weights(lora_A, lora_B)
            in_maps, splits = _make_chunk_in_maps(x2, twf, chunk, a_p, b_p)
            res = run_bass_kernel_spmd(nc, in_maps, list(range(B_CORES)))
            for i, ids in enumerate(splits):
                if len(ids):
                    out[ids] = res.results[i]["out"][: len(ids)].astype(np.float32)
            pos += len(chunk)

    if pos < len(idx):
        ids = idx[pos:]
        a32 = np.asarray(lora_A, dtype=np.float32)
        b32 = np.asarray(lora_B, dtype=np.float32)
        xs = x2[ids] * (LORA_SCALING * twf[ids])[:, None]
        out[ids] = (xs @ a32) @ b32

    return out.reshape(B_CORES, S, D)


if __name__ == "__main__":
    nc = get_bass()
    print("built + compiled ok")


# revision 32
# speedup vs baseline: 1.1069x; 1.0215x over previous
"""Trainium2 Bass kernel for a gated LoRA adapter layer (MoE-style routing).

Computes, for x:(8,2048,4096) f32, type_weight:(8,2048) f32,
lora_A:(4096,64) f32, lora_B:(64,4096) f32:

    out = type_weight[..., None] * ((x @ lora_A) @ lora_B) * 2.0

Routing insight: ~50% of tokens have type_weight == 0 and contribute an
exactly-zero output row.  The host compacts the nonzero tokens (the
"router"), folds the gate into x (x_row * 2*tw), pre-transposes so the
contraction dim lands on partitions, and casts everything to bf16.  The
8 cores each run a dense (x.T-major) LoRA on exactly 1024 tokens (two
512-token stages); the device capacity is 8*1024 = 8192 global tokens —
right at the mean nonzero count — and any overflow tokens (mean ~25,
std ~64 for Bernoulli(0.5) gates) are computed exactly on the host in
f32 numpy.  Outputs are stored bf16 and scattered into the
zero-initialized full f32 result.

Device pipeline per core:
  - mm1: t.T = sum_dt A[dt].T @ xT[dt], with A's columns duplicated so the
    [128, 512] PSUM result holds t.T on partitions 0-63 AND 64-127.
  - mm2: out row-blocks via PAIRED matmuls in disjoint PE row groups
    (rows 0-63 / 64-127, K=64 each) -> 2 concurrent MMs per issue, into
    one [128, 1024] two-bank PSUM tile drained by a single copy.
  - mm1 of stage 1 is interleaved between mm2 slots of stage 0 so orows
    start flowing early and the PE fills the copy-drain time.
  - B is duplicated to partitions 64-127 on-device (SBUF->SBUF DMA).
"""

import numpy as np
import ml_dtypes

BF16 = ml_dtypes.bfloat16

B_CORES = 8
S = 2048
D = 4096
R = 64
LORA_SCALING = 128.0 / 64.0

T_STAGE = 256
N_STAGES = 4
S_PAD = T_STAGE * N_STAGES  # 1024 per-core device capacity
N_DT = D // 128             # 32 d-tiles
N_DC = D // 512             # 8 output column chunks
N_ST = T_STAGE // 128       # 4 output row blocks per stage (2 pairs)
HOST_OVERFLOW_MAX = 2048    # beyond this, loop more device runs

_CACHE = {}

# osb tiles are [128, 2, D] pairs (16KB/partition); ps_o tiles are single
# PSUM banks ([128, 512] f32) - 6 bufs = 3 pair-slots of pipeline depth.
OPTS = {
    "x_bufs": 3,
    "osb_bufs": 5,
    "ps_t_bufs": 2,
    "ps_o_bufs": 6,
}


def _build_bass():
    import concourse.tile as tile
    from concourse import bacc, mybir

    nc = bacc.Bacc(
        "TRN2",
        debug=False,
        enable_asserts=False,
        target_bir_lowering=False,
        num_devices=B_CORES,
    )

    f32 = mybir.dt.float32
    bf16 = mybir.dt.bfloat16

    # Host-prepped layouts (see _prep_core / _prep_weights):
    #   x:  [128, 32*S_PAD]  stage-major [p][j][dt][s], d = dt*128 + p
    #   a:  [128, N_DT * R]  = [p][dt][r]
    #   b:  [R, D]
    x_d = nc.dram_tensor("x", [128, N_DT * S_PAD], bf16, kind="ExternalInput").ap()
    a_d = nc.dram_tensor("lora_a", [128, N_DT * R], bf16, kind="ExternalInput").ap()
    b_d = nc.dram_tensor("lora_b", [2 * R, D], bf16, kind="ExternalInput").ap()
    out_d = nc.dram_tensor("out", [S_PAD, D], bf16, kind="ExternalOutput").ap()

    with tile.TileContext(nc) as tc:
        with (
            tc.tile_pool(name="consts", bufs=1) as consts,
            tc.tile_pool(name="xsb", bufs=OPTS["x_bufs"]) as xsb,
            tc.tile_pool(name="ttp", bufs=2) as ttp,
            tc.tile_pool(name="osb", bufs=OPTS["osb_bufs"]) as osb,
            tc.tile_pool(name="ps_t", bufs=OPTS["ps_t_bufs"], space="PSUM") as ps_t,
            tc.tile_pool(name="ps_o", bufs=OPTS["ps_o_bufs"], space="PSUM") as ps_o,
        ):
            # A with duplicated columns: a_sb[p, dt, 0:64] == a_sb[p, dt, 64:128]
            # == A[dt*128+p, :].  Load once, duplicate with DVE copies.
            a_tmp = consts.tile([128, N_DT, R], bf16)
            nc.sync.dma_start(a_tmp[:], a_d.rearrange("p (dt r) -> p dt r", r=R))
            a_sb = consts.tile([128, N_DT, 2 * R], bf16)
            nc.vector.tensor_copy(a_sb[:, :, 0:R], a_tmp[:])
            nc.vector.tensor_copy(a_sb[:, :, R : 2 * R], a_tmp[:])

            # B arrives host-duplicated to 128 partitions (an on-device
            # SBUF->SBUF SWDGE duplicate measured ~10us late on the critical
            # path — every B-row-group matmul gated on it).
            b_sb = consts.tile([128, D], bf16)

            # x stage loads in quarters so mm1 chases the FIFO-ordered loads;
            # B slots in after stage 0 so mm2(0) can start.
            xts = []
            for j in range(N_STAGES):
                xt = xsb.tile([128, N_DT, T_STAGE], bf16, name=f"xt{j}", tag="xt")
                for h in range(4):
                    off = (j * N_DT + h * 8) * T_STAGE
                    src = x_d[:, off : off + 8 * T_STAGE].rearrange(
                        "p (dt s) -> p dt s", s=T_STAGE
                    )
                    nc.sync.dma_start(xt[:, h * 8 : (h + 1) * 8, :], src)
                if j == 0:
                    nc.sync.dma_start(b_sb[:], b_d)
                xts.append(xt)

            def emit_mm1(j, ps, dt):
                # t.T (duplicated over both partition halves) accumulated f32.
                nc.tensor.matmul(
                    ps[:],
                    lhsT=a_sb[:, dt, :],
                    rhs=xts[j][:, dt, :],
                    start=(dt == 0),
                    stop=(dt == N_DT - 1),
                )

            def emit_mm2_stage(j, ttj, interleave):
                """mm2 slots for stage j; optionally interleave (fn per slot)."""
                for q in range(N_ST // 2):
                    stA, stB = 2 * q, 2 * q + 1
                    orow2 = osb.tile(
                        [128, 2, D], bf16, name=f"or2_{j}_{q}", tag="orow"
                    )
                    r0 = (j * N_ST + stA) * 128
                    for dc in range(N_DC):
                        cs = slice(dc * 512, (dc + 1) * 512)
                        psoA = ps_o.tile([128, 512], f32, name="psoA", tag="pso")
                        nc.tensor.matmul(
                            psoA[:],
                            lhsT=ttj[0:R, stA * 128 : (stA + 1) * 128],
                            rhs=b_sb[0:R, cs],
                            start=True,
                            stop=True,
                        )
                        psoB = ps_o.tile([128, 512], f32, name="psoB", tag="pso")
                        nc.tensor.matmul(
                            psoB[:],
                            lhsT=ttj[R : 2 * R, stB * 128 : (stB + 1) * 128],
                            rhs=b_sb[R : 2 * R, cs],
                            start=True,
                            stop=True,
                        )
                        if interleave is not None:
                            interleave(q * N_DC + dc)
                        nc.vector.tensor_copy(orow2[:, 0, cs], psoA[:])
                        nc.scalar.copy(orow2[:, 1, cs], psoB[:])
                        # Store each 1024-col pair as soon as its drains land,
                        # split across both DMA rings (ACT + the SP ring that
                        # sits idle once loads finish) so the store stream never
                        # waits for a full row-block and the tail store shrinks.
                        if dc % 2 == 1:
                            cs2 = slice((dc - 1) * 512, (dc + 1) * 512)
                            nc.scalar.dma_start(
                                out_d[r0 : r0 + 128, cs2], orow2[:, 0, cs2]
                            )
                            nc.sync.dma_start(
                                out_d[r0 + 128 : r0 + 256, cs2], orow2[:, 1, cs2]
                            )

            # Pipelined stages: mm1(0) dense (chasing x0's quarter loads),
            # then for each stage j: mm2(j) with mm1(j+1) interleaved so every
            # stage's t.T is ready the moment its mm2 begins.  Short stages
            # mean mm2/stores start ~12us in, overlapping the load phase.
            ILV = -(-N_DT // ((N_ST // 2) * N_DC))  # mm1 MMs per mm2 slot
            tts = [None] * N_STAGES
            pss = [None] * N_STAGES
            pss[0] = ps_t.tile([128, T_STAGE], f32, name="psmm1_0", tag="mm1ps")
            for dt in range(N_DT):
                emit_mm1(0, pss[0], dt)
            tts[0] = ttp.tile([128, T_STAGE], bf16, name="tt0", tag="tt")
            nc.vector.tensor_copy(tts[0][:], pss[0][:])

            for j in range(N_STAGES):
                if j + 1 < N_STAGES:
                    pss[j + 1] = ps_t.tile(
                        [128, T_STAGE], f32, name=f"psmm1_{j + 1}", tag="mm1ps"
                    )

                    def ilv(slot, jn=j + 1):
                        for dt in range(ILV * slot, ILV * (slot + 1)):
                            if dt < N_DT:
                                emit_mm1(jn, pss[jn], dt)

                    emit_mm2_stage(j, tts[j], ilv)
                    tts[j + 1] = ttp.tile(
                        [128, T_STAGE], bf16, name=f"tt{j + 1}", tag="tt"
                    )
                    nc.vector.tensor_copy(tts[j + 1][:], pss[j + 1][:])
                else:
                    emit_mm2_stage(j, tts[j], None)

    nc.compile()
    return nc


def get_bass():
    if "nc" not in _CACHE:
        _CACHE["nc"] = _build_bass()
    return _CACHE["nc"]


def _prep_weights(lora_A, lora_B):
    a = np.asarray(lora_A, dtype=np.float32).astype(BF16)
    # [D, R] -> [p][dt][r] with d = dt*128 + p
    a_p = np.ascontiguousarray(a.reshape(N_DT, 128, R).transpose(1, 0, 2)).reshape(
        128, N_DT * R
    )
    b = np.asarray(lora_B, dtype=np.float32).astype(BF16)
    b_p = np.ascontiguousarray(np.concatenate([b, b], axis=0))  # [2R, D]
    return a_p, b_p


def _prep_core(x2, scale, ids):
    """Gather + gate-fold + pad + transpose one core's tokens.

    Returns [128, N_DT*S_PAD] bf16, stage-major [p][j][dt][s]."""
    n = len(ids)
    xsb = np.zeros((S_PAD, D), dtype=BF16)
    if n:
        xsb[:n] = (x2[ids] * scale[:, None]).astype(BF16)
    blk = xsb.reshape(N_STAGES, T_STAGE, N_DT, 128).transpose(3, 0, 2, 1)
    return np.ascontiguousarray(blk).reshape(128, N_DT * S_PAD)


def _make_chunk_in_maps(x2, twf, idx_chunk, a_p, b_p):
    splits = np.array_split(idx_chunk, B_CORES)
    in_maps = []
    for ids in splits:
        scale = LORA_SCALING * twf[ids]
        in_maps.append(
            {
                "x": _prep_core(x2, scale, ids),
                "lora_a": a_p,
                "lora_b": b_p,
            }
        )
    return in_maps, splits


def make_in_maps(x, type_weight, lora_A, lora_B):
    """First-chunk in_maps (what kernel() runs on the device)."""
    x2 = np.asarray(x, dtype=np.float32).reshape(B_CORES * S, D)
    twf = np.asarray(type_weight, dtype=np.float32).reshape(B_CORES * S)
    idx = np.flatnonzero(twf)[: B_CORES * S_PAD]
    a_p, b_p = _prep_weights(lora_A, lora_B)
    in_maps, _ = _make_chunk_in_maps(x2, twf, idx, a_p, b_p)
    return in_maps


def kernel(x, type_weight, lora_A, lora_B):
    from concourse.bass_utils import run_bass_kernel_spmd

    x2 = np.asarray(x, dtype=np.float32).reshape(B_CORES * S, D)
    twf = np.asarray(type_weight, dtype=np.float32).reshape(B_CORES * S)
    out = np.zeros((B_CORES * S, D), dtype=np.float32)

    idx = np.flatnonzero(twf)
    cap = B_CORES * S_PAD
    pos = 0
    if len(idx):
        # Device runs on chunks of `cap` tokens while the remainder is large;
        # the final small overflow (mean ~25 tokens for 50%-sparse gates) is
        # computed exactly on the host instead of paying another device run.
        a_p = b_p = None
        while len(idx) - pos > HOST_OVERFLOW_MAX or (pos == 0 and len(idx) - pos > 0):
            chunk = idx[pos : pos + cap]
            if a_p is None:
                nc = get_bass()
                a_p, b_p = _prep_weights(lora_A, lora_B)
            in_maps, splits = _make_chunk_in_maps(x2, twf, chunk, a_p, b_p)
            res = run_bass_kernel_spmd(nc, in_maps, list(range(B_CORES)))
            for i, ids in enumerate(splits):
                if len(ids):
                    out[ids] = res.results[i]["out"][: len(ids)].astype(np.float32)
            pos += len(chunk)

    if pos < len(idx):
        ids = idx[pos:]
        a32 = np.asarray(lora_A, dtype=np.float32)
        b32 = np.asarray(lora_B, dtype=np.float32)
        xs = x2[ids] * (LORA_SCALING * twf[ids])[:, None]
        out[ids] = (xs @ a32) @ b32

    return out.reshape(B_CORES, S, D)


if __name__ == "__main__":
    nc = get_bass()
    print("built + compiled ok")
